# revision 23
# baseline (speedup 1.0000x reference)
"""Trainium2 Bass kernel for bidirectional GRU (nn_Bidirectional) — v4.

Model: y = BN2(concat([GRU_f(BN1(x@w_in)), rev(GRU_b(rev(BN1(x@w_in))))]) @ w_out)
Shapes: x [64, 512, 128], H=512, O=8.

Sharding: 8 cores = 2 directions x 4 cores; the time axis is cut into 8
sub-segments of 60 output steps (seg0: 92). The GRU forgets its initial
state within ~32 steps (measured restart error ~2e-6), so every sub-segment
s>0 starts 32 steps early from h=0. Each core runs TWO chains (sub-segments
2i and 2i+1) over the FULL batch B=64, interleaved step-by-step: while
chain A's gate chain (ACT/DVE ops, ~2us serial latency) runs, the PE
executes chain B's matmul block, so the PE never waits on the recurrence
nonlinearities. 92 steps per chain, 184 per core.

Device program per core (feature-major [unit, token] layout):
  - xp GEMM: xp = x @ Wcomb + bxp, with Wcomb = (w_in*s1) @ wx host-fused
    (BN1 folded, contraction 128), emitted a few N=512 matmuls per step as
    extra PE filler; results land in per-chain SBUF rings (no DRAM scratch).
  - recurrence step: [id-matmuls (add xz/xr into PSUM, no h dependency) |
    r-gate MMs (own accumulation group -> sig_r fires mid-block) | h-gate
    MMs | z-gate MMs], then gate chain t1=ps_h*r -> t2=t1+xh -> hh=tanh(t2)
    -> dd=h-hh -> ee=z*dd -> h'=ee+hh. z/r recurrent weights fp8-e4m3
    (end-to-end rel err 0.0076 measured), candidate weights bf16.
  - y projection: per chunk, h history (SBUF ring) @ wo_half -> yT DRAM.
"""

import sys
from contextlib import ExitStack

import numpy as np
import ml_dtypes

if "/opt/trn_rl_repo" not in sys.path:
    sys.path.insert(0, "/opt/trn_rl_repo")

B, T, F, H, O = 64, 512, 128, 512, 8
EPS = 1e-3
NCORES = 8
KT = H // 128          # 4 k-strips
MT = 3 * H // 128      # 12 xp strips (z0..3, r0..3, h0..3)
W = 8                  # warm-up steps
NSEG = 8               # time sub-segments (2 per core)
L8 = (T - W) // NSEG   # 63 output steps per sub-segment (seg0: 71)
P = L8 + W             # 71 steps per chain
SPC = 8                # steps per full chunk
CH = SPC * B           # 512 tokens per chunk
NCH = 9                # chunks per chain (8 full + 1 partial of 7 steps)
SPC_LAST = P - 8 * SPC   # 7
NCHP = NCH + 2         # padded chunks in xT (GEMM lookahead)
CTOK = P * B           # 5888 real tokens per chain
BF16 = ml_dtypes.bfloat16
FP8 = ml_dtypes.float8_e4m3

# xh GEMM (m, chain) list per dual-chunk: 8 MMs spread over 8 dual-steps
_MLIST = [(m, a) for m in range(8, MT) for a in (0, 1)]
M_SCHED = [_MLIST[j:j + 1] for j in range(8)]

_cache = {}


def _build(has_bh: bool):
    import concourse.bass as bass
    import concourse.bacc as bacc
    import concourse.tile as tile
    import concourse.mybir as mybir

    dt = mybir.dt
    f32 = dt.float32
    bf = dt.bfloat16
    f8 = dt.float8e4
    AF = mybir.ActivationFunctionType
    OP = mybir.AluOpType
    ds = bass.ds

    nc = bacc.Bacc("TRN2", target_bir_lowering=False, debug=False,
                   num_devices=NCORES)

    xT = nc.dram_tensor("xT", [128, 2, NCHP * CH], bf, kind="ExternalInput").ap()
    wcomb = nc.dram_tensor("wcomb", [128, MT, 128], bf, kind="ExternalInput").ap()
    bxp = nc.dram_tensor("bxp", [128, MT], f32, kind="ExternalInput").ap()
    wh8 = nc.dram_tensor("wh8", [128, KT, 8, 128], f8, kind="ExternalInput").ap()
    whh = nc.dram_tensor("whh", [128, KT, 4, 128], bf, kind="ExternalInput").ap()
    bhr = nc.dram_tensor("bhr", [128, KT], f32, kind="ExternalInput").ap()
    ident = nc.dram_tensor("ident", [128, 128], bf, kind="ExternalInput").ap()
    bzr = nc.dram_tensor("bzr", [128, 2, KT, B], bf, kind="ExternalInput").ap()
    wo = nc.dram_tensor("wo", [128, KT, O], bf, kind="ExternalInput").ap()
    bo = nc.dram_tensor("bo", [O, 1], f32, kind="ExternalInput").ap()
    yT = nc.dram_tensor("yT", [O, 2, NCH * CH], f32, kind="ExternalOutput").ap()

    with tile.TileContext(nc) as tc, ExitStack() as ctx:
        consts = ctx.enter_context(tc.tile_pool(name="consts", bufs=1))
        big = ctx.enter_context(tc.tile_pool(name="big", bufs=1))
        stg = ctx.enter_context(tc.tile_pool(name="stg", bufs=3))
        gates = ctx.enter_context(tc.tile_pool(name="gates", bufs=2))
        psA = ctx.enter_context(tc.tile_pool(name="psA", bufs=1, space="PSUM"))
        psXP = ctx.enter_context(tc.tile_pool(name="psXP", bufs=2, space="PSUM"))
        psYp = ctx.enter_context(tc.tile_pool(name="psYp", bufs=2, space="PSUM"))

        # ---------- constants ----------
        wcomb_sb = consts.tile([128, MT, 128], bf)
        nc.sync.dma_start(out=wcomb_sb, in_=wcomb)
        bxp_sb = consts.tile([128, MT], f32)
        nc.sync.dma_start(out=bxp_sb, in_=bxp)
        wh8_sb = consts.tile([128, KT, 8, 128], f8)
        nc.sync.dma_start(out=wh8_sb, in_=wh8)
        whh_sb = consts.tile([128, KT, 4, 128], bf)
        nc.sync.dma_start(out=whh_sb, in_=whh)
        bhr_sb = consts.tile([128, KT], f32)
        nc.sync.dma_start(out=bhr_sb, in_=bhr)
        id_sb = consts.tile([128, 128], bf)
        nc.sync.dma_start(out=id_sb, in_=ident)
        bzr_sb = consts.tile([128, 2, KT, B], bf)
        nc.sync.dma_start(out=bzr_sb, in_=bzr)
        wo_sb = consts.tile([128, KT, O], bf)
        nc.sync.dma_start(out=wo_sb, in_=wo)
        bo_sb = consts.tile([O, 1], f32)
        nc.sync.dma_start(out=bo_sb, in_=bo)

        # ---------- per-chain rings ----------
        xps = [[big.tile([128, 4, CH], bf, tag=f"xp{a}{r}", name=f"xp{a}{r}")
                for r in range(3)] for a in range(2)]
        hist = [[big.tile([128, KT, CH], bf, tag=f"hi{a}{r}", name=f"hi{a}{r}")
                 for r in range(3)] for a in range(2)]
        xstg = [[big.tile([128, CH], bf, tag=f"xs{a}{r}", name=f"xs{a}{r}")
                 for r in range(3)] for a in range(2)]
        hz = big.tile([128, KT, B], bf, tag="hz")
        nc.vector.memset(hz, 0.0)

        def xp_mm(a, gemm_slot, m):
            """One xh GEMM matmul + biased copy into chain a's xp ring."""
            ps = psXP.tile([128, CH], f32, tag="xp")
            nc.tensor.matmul(ps, wcomb_sb[:, m, :], xstg[a][gemm_slot],
                             start=True, stop=True)
            if m % 2 == 0:
                nc.scalar.activation(xps[a][gemm_slot][:, m - 8, :], ps,
                                     AF.Identity, bias=bxp_sb[:, m:m + 1],
                                     scale=1.0)
            else:
                nc.vector.tensor_scalar_add(xps[a][gemm_slot][:, m - 8, :], ps,
                                            bxp_sb[:, m:m + 1])

        def step(a, slot, j, h_in):
            """One recurrence step of chain a (full batch, free dim 64)."""
            xp_c = xps[a][slot]
            tk = j * B
            xh = xp_c[:, 0:4, tk:tk + B]
            x_t = xstg[a][slot][:, tk:tk + B]
            h_out = hist[a][slot][:, :, tk:tk + B]

            psRr = psA.tile([128, KT, B], f32, tag="r")
            psZ = psA.tile([128, KT, B], f32, tag="z", bufs=2)
            psH = psA.tile([128, KT, B], f32, tag="h")

            # bias + x-projection into PSUM; no h dependency -- the PE runs
            # these during the other chain's gate chain.
            nc.tensor.matmul(psRr, id_sb, bzr_sb[:, 0], start=True, stop=False)
            nc.tensor.matmul(psZ, id_sb, bzr_sb[:, 1], start=True, stop=False)
            for s in range(KT):
                nc.tensor.matmul(psRr[:, s], wcomb_sb[:, 4 + s, :], x_t,
                                 start=False, stop=False)
                nc.tensor.matmul(psZ[:, s], wcomb_sb[:, s, :], x_t,
                                 start=False, stop=False)
            # r first in its own group (sig_r fires mid-block), then h, z last.
            for s in range(KT):
                for k in range(KT):
                    nc.tensor.matmul(psRr[:, s], wh8_sb[:, k, 4 + s, :],
                                     h_in[:, k], start=False,
                                     stop=(s == KT - 1 and k == KT - 1))
            for s in range(KT):
                for k in range(KT):
                    nc.tensor.matmul(psH[:, s], whh_sb[:, k, s, :],
                                     h_in[:, k], start=(k == 0),
                                     stop=(k == KT - 1))
            for s in range(KT):
                for k in range(KT):
                    nc.tensor.matmul(psZ[:, s], wh8_sb[:, k, s, :],
                                     h_in[:, k], start=False,
                                     stop=(s == KT - 1 and k == KT - 1))

            r_sb = gates.tile([128, KT, B], bf, tag="r")
            nc.scalar.activation(r_sb, psRr, AF.Sigmoid)
            z_sb = gates.tile([128, KT, B], bf, tag="z")
            nc.scalar.activation(z_sb, psZ, AF.Sigmoid)
            t1 = gates.tile([128, KT, B], bf, tag="t1")
            if has_bh:
                for s in range(KT):
                    nc.vector.scalar_tensor_tensor(
                        t1[:, s], psH[:, s], bhr_sb[:, s:s + 1], r_sb[:, s],
                        OP.add, OP.mult)
            else:
                nc.vector.tensor_mul(t1, psH, r_sb)
            t2 = gates.tile([128, KT, B], bf, tag="t2")
            nc.vector.tensor_add(t2, t1, xh)
            hh = gates.tile([128, KT, B], bf, tag="hh")
            nc.scalar.activation(hh, t2, AF.Tanh)
            dd = gates.tile([128, KT, B], bf, tag="dd")
            nc.vector.tensor_sub(dd, h_in, hh)
            ee = gates.tile([128, KT, B], bf, tag="ee")
            nc.vector.tensor_mul(ee, z_sb, dd)
            nc.vector.tensor_add(h_out, ee, hh)
            return h_out

        def dual_chunk(c_slot, prev_slot, gemm_slot, hps, nsteps=SPC,
                       first_chunk=False, y_prev=None):
            """nsteps x 2 chains, interleaved; xp GEMM and the previous
            chunk's y-projection matmuls spread out as PE gap filler."""
            for a in range(2):
                if first_chunk:
                    hps[a] = hz
                elif hps[a] is None:
                    hps[a] = hist[a][prev_slot][:, :, (SPC - 1) * B:SPC * B]
            yps = [None, None]

            def y_tick(j):
                ys, yc = y_prev
                a, k = (0, j) if j < 4 else (1, j - 4)
                if k == 0:
                    yps[a] = psYp.tile([O, CH], f32, tag="y", name="psY")
                nc.tensor.matmul(yps[a], wo_sb[:, k, :], hist[a][ys][:, k, :],
                                 start=(k == 0), stop=(k == 3))
                if k == 3:
                    yst = stg.tile([O, CH], f32, tag="yst", name="yst")
                    nc.scalar.activation(yst, yps[a], AF.Identity,
                                         bias=bo_sb, scale=1.0)
                    nc.sync.dma_start(out=yT[:, a, ds(yc * CH, CH)], in_=yst)

            for j in range(nsteps):
                hps[0] = step(0, c_slot, j, hps[0])
                hps[1] = step(1, c_slot, j, hps[1])
                if gemm_slot is not None:
                    for m, a in M_SCHED[j]:
                        xp_mm(a, gemm_slot, m)
                if y_prev is not None and j < 8:
                    y_tick(j)
            if y_prev is not None:
                for j in range(nsteps, 8):
                    y_tick(j)
            return hps

        def y_gemm(a, c_slot, c_expr):
            psY = psYp.tile([O, CH], f32, tag="y")
            for k in range(KT):
                nc.tensor.matmul(psY, wo_sb[:, k, :], hist[a][c_slot][:, k, :],
                                 start=(k == 0), stop=(k == KT - 1))
            yst = stg.tile([O, CH], f32, tag="yst")
            nc.scalar.activation(yst, psY, AF.Identity, bias=bo_sb, scale=1.0)
            nc.sync.dma_start(out=yT[:, a, ds(c_expr * CH, CH)], in_=yst)

        # ---------- prologue: xp chunks 0/1 both chains, chunk 0 steps ----
        for a in range(2):
            nc.sync.dma_start(out=xstg[a][0], in_=xT[:, a, 0:CH])
            nc.sync.dma_start(out=xstg[a][1], in_=xT[:, a, CH:2 * CH])
        for a in range(2):
            for m in range(8, MT):
                xp_mm(a, 0, m)
            for m in range(8, MT):
                xp_mm(a, 1, m)
        for a in range(2):
            nc.sync.dma_start(out=xstg[a][2], in_=xT[:, a, 2 * CH:3 * CH])
        hps = [None, None]
        hps = dual_chunk(0, None, 2, hps, first_chunk=True)

        # ---------- main chunks 1..6, fully unrolled ----------
        for i in (1, 4):
            # chunk i -> slot 1, i+1 -> slot 2, i+2 -> slot 0
            for a in range(2):
                nc.sync.dma_start(out=xstg[a][0],
                                  in_=xT[:, a, ds((i + 2) * CH, CH)])
            hps = dual_chunk(1, 0, 0, [None, None], y_prev=((i - 1) % 3, i - 1))
            for a in range(2):
                nc.sync.dma_start(out=xstg[a][1],
                                  in_=xT[:, a, ds((i + 3) * CH, CH)])
            hps = dual_chunk(2, 1, 1, [None, None], y_prev=(i % 3, i))
            for a in range(2):
                nc.sync.dma_start(out=xstg[a][2],
                                  in_=xT[:, a, ds((i + 4) * CH, CH)])
            hps = dual_chunk(0, 2, 2, [None, None],
                             y_prev=((i + 1) % 3, i + 1))

        # ---------- epilogue: chunk 7 (full), 8 (partial) ----------
        for a in range(2):
            nc.sync.dma_start(out=xstg[a][0], in_=xT[:, a, 9 * CH:10 * CH])
        hps = dual_chunk(1, 0, 0, [None, None], y_prev=(0, 6))  # chunk 7
        hps = dual_chunk(2, 1, None, [None, None], nsteps=SPC_LAST,
                         y_prev=(1, 7))                          # chunk 8
        y_gemm(0, 2, 8)
        y_gemm(1, 2, 8)

    nc.compile()
    return nc


def _get_program(has_bh: bool):
    key = ("prog", has_bh)
    if key not in _cache:
        _cache[key] = _build(has_bh)
    return _cache[key]


def _chain_tokens(x, dirn, seg):
    """[128, NCHP*CH] bf16 token stream for one sub-segment chain."""
    t0 = seg * L8
    tsel = np.arange(t0, t0 + P)
    tglob = tsel if dirn == 0 else (T - 1 - tsel)
    xs = x[:, tglob, :]                                   # [B,P,F]
    out = np.zeros((128, NCHP * CH), np.float32)
    out[:, :CTOK] = xs.transpose(2, 1, 0).reshape(F, CTOK)
    return out


def _prep_core(x, dirn, i, wcomb_bf, bxp_f, wh, bb, wo_half, bias_out):
    """Per-core input map. x is the full [B,T,F] fp32 array."""
    xTc = np.stack([_chain_tokens(x, dirn, 2 * i),
                    _chain_tokens(x, dirn, 2 * i + 1)], axis=1)
    whr = wh.reshape(KT, 128, MT, 128).transpose(1, 0, 2, 3)  # [kp,ks,m,p]
    return {
        "xT": xTc.astype(BF16),
        "wcomb": wcomb_bf,
        "bxp": bxp_f,
        "wh8": np.ascontiguousarray(whr[:, :, 0:8]).astype(FP8),
        "whh": np.ascontiguousarray(whr[:, :, 8:12]).astype(BF16),
        "bhr": np.ascontiguousarray(
            bb[1, 2 * H:].reshape(KT, 128).T.astype(np.float32)),
        "ident": np.eye(128).astype(BF16),
        "bzr": np.ascontiguousarray(np.broadcast_to(
            np.stack([bxp_f[:, 4:8], bxp_f[:, 0:4]], axis=1)[:, :, :, None],
            (128, 2, KT, B))).astype(BF16),
        "wo": np.ascontiguousarray(
            wo_half.reshape(KT, 128, O).transpose(1, 0, 2)).astype(BF16),
        "bo": bias_out.reshape(O, 1).astype(np.float32),
    }


def _prepare(np_in):
    """Build (nc, in_maps) for the 8 cores."""
    s1 = np_in["g1"] / np.sqrt(np_in["v1"] + EPS)
    b1 = (np_in["b_in"] - np_in["m1"]) * s1 + np_in["be1"]
    s2 = np_in["g2"] / np.sqrt(np_in["v2"] + EPS)
    b2 = (np_in["b_out"] - np_in["m2"]) * s2 + np_in["be2"]
    Ws = np_in["w_out"] * s2[None, :]

    has_bh = bool(np.any(np_in["bf"][1, 2 * H:]) or np.any(np_in["bb"][1, 2 * H:]))
    nc = _get_program(has_bh)

    in_maps = []
    for c in range(NCORES):
        dirn, i = c // 4, c % 4
        wx = np_in["wxf"] if dirn == 0 else np_in["wxb"]
        wh = np_in["whf"] if dirn == 0 else np_in["whb"]
        bb = np_in["bf"] if dirn == 0 else np_in["bb"]
        wcomb = ((np_in["w_in"] * s1[None, :]) @ wx).astype(np.float32)
        wcomb_bf = np.ascontiguousarray(
            wcomb.reshape(128, MT, 128)).astype(BF16)
        bxp_full = (b1 @ wx + bb[0]
                    + np.concatenate([bb[1, :2 * H], np.zeros(H, np.float32)]))
        bxp_f = np.ascontiguousarray(
            bxp_full.reshape(MT, 128).T.astype(np.float32))
        wo_half = Ws[:H] if dirn == 0 else Ws[H:]
        bias_o = b2 if dirn == 0 else np.zeros(O, np.float32)
        in_maps.append(_prep_core(np_in["x"], dirn, i, wcomb_bf, bxp_f,
                                  wh, bb, wo_half, bias_o))
    return nc, in_maps


def _assemble(outs):
    """Sum per-core yT partials into the full [B,T,O] output."""
    y = np.zeros((B, T, O), np.float32)
    for c in range(NCORES):
        dirn, i = c // 4, c % 4
        yc = outs[c]["yT"].reshape(O, 2, NCH * CH)
        for a in range(2):
            seg = 2 * i + a
            t0 = seg * L8
            tsel = np.arange(t0, t0 + P)
            tglob = tsel if dirn == 0 else (T - 1 - tsel)
            k0 = 0 if seg == 0 else W
            ya = yc[:, a, :CTOK].reshape(O, P, B)
            y[:, tglob[k0:], :] += ya[:, k0:, :].transpose(2, 1, 0)
    return y


def kernel(x, w_in, b_in, g1, be1, m1, v1, wxf, whf, bf, wxb, whb, bb,
           w_out, b_out, g2, be2, m2, v2):
    from concourse.bass_utils import run_bass_kernel_spmd

    args = locals()
    np_in = {k: np.asarray(args[k], np.float32) for k in (
        "x", "w_in", "b_in", "g1", "be1", "m1", "v1", "wxf", "whf", "bf",
        "wxb", "whb", "bb", "w_out", "b_out", "g2", "be2", "m2", "v2")}
    nc, in_maps = _prepare(np_in)
    res = run_bass_kernel_spmd(nc, in_maps, core_ids=list(range(NCORES)))
    return _assemble(res.results)


# revision 24
# speedup vs baseline: 1.0187x; 1.0187x over previous
"""Trainium2 Bass kernel for bidirectional GRU (nn_Bidirectional) — v4.

Model: y = BN2(concat([GRU_f(BN1(x@w_in)), rev(GRU_b(rev(BN1(x@w_in))))]) @ w_out)
Shapes: x [64, 512, 128], H=512, O=8.

Sharding: 8 cores = 2 directions x 4 cores; the time axis is cut into 8
sub-segments of 60 output steps (seg0: 92). The GRU forgets its initial
state within ~32 steps (measured restart error ~2e-6), so every sub-segment
s>0 starts 32 steps early from h=0. Each core runs TWO chains (sub-segments
2i and 2i+1) over the FULL batch B=64, interleaved step-by-step: while
chain A's gate chain (ACT/DVE ops, ~2us serial latency) runs, the PE
executes chain B's matmul block, so the PE never waits on the recurrence
nonlinearities. 92 steps per chain, 184 per core.

Device program per core (feature-major [unit, token] layout):
  - xp GEMM: xp = x @ Wcomb + bxp, with Wcomb = (w_in*s1) @ wx host-fused
    (BN1 folded, contraction 128), emitted a few N=512 matmuls per step as
    extra PE filler; results land in per-chain SBUF rings (no DRAM scratch).
  - recurrence step: [id-matmuls (add xz/xr into PSUM, no h dependency) |
    r-gate MMs (own accumulation group -> sig_r fires mid-block) | h-gate
    MMs | z-gate MMs], then gate chain t1=ps_h*r -> t2=t1+xh -> hh=tanh(t2)
    -> dd=h-hh -> ee=z*dd -> h'=ee+hh. z/r recurrent weights fp8-e4m3
    (end-to-end rel err 0.0076 measured), candidate weights bf16.
  - y projection: per chunk, h history (SBUF ring) @ wo_half -> yT DRAM.
"""

import sys
from contextlib import ExitStack

import numpy as np
import ml_dtypes

if "/opt/trn_rl_repo" not in sys.path:
    sys.path.insert(0, "/opt/trn_rl_repo")

B, T, F, H, O = 64, 512, 128, 512, 8
EPS = 1e-3
NCORES = 8
KT = H // 128          # 4 k-strips
MT = 3 * H // 128      # 12 xp strips (z0..3, r0..3, h0..3)
W = 8                  # warm-up steps
NSEG = 8               # time sub-segments (2 per core)
L8 = (T - W) // NSEG   # 63 output steps per sub-segment (seg0: 71)
P = L8 + W             # 71 steps per chain
SPC = 8                # steps per full chunk
CH = SPC * B           # 512 tokens per chunk
NCH = 9                # chunks per chain (8 full + 1 partial of 7 steps)
SPC_LAST = P - 8 * SPC   # 7
NCHP = NCH + 2         # padded chunks in xT (GEMM lookahead)
CTOK = P * B           # 5888 real tokens per chain
BF16 = ml_dtypes.bfloat16
FP8 = ml_dtypes.float8_e4m3

# xh GEMM (m, chain) list per dual-chunk: 8 MMs spread over 8 dual-steps
_MLIST = [(m, a) for m in range(8, MT) for a in (0, 1)]
M_SCHED = [_MLIST[j:j + 1] for j in range(8)]

_cache = {}


def _build(has_bh: bool):
    import concourse.bass as bass
    import concourse.bacc as bacc
    import concourse.tile as tile
    import concourse.mybir as mybir

    dt = mybir.dt
    f32 = dt.float32
    bf = dt.bfloat16
    f8 = dt.float8e4
    AF = mybir.ActivationFunctionType
    OP = mybir.AluOpType
    ds = bass.ds

    nc = bacc.Bacc("TRN2", target_bir_lowering=False, debug=False,
                   num_devices=NCORES)

    xT = nc.dram_tensor("xT", [128, 2, NCHP * CH], bf, kind="ExternalInput").ap()
    wcomb = nc.dram_tensor("wcomb", [128, MT, 128], bf, kind="ExternalInput").ap()
    bxp = nc.dram_tensor("bxp", [128, MT], f32, kind="ExternalInput").ap()
    wh8 = nc.dram_tensor("wh8", [128, KT, 8, 128], f8, kind="ExternalInput").ap()
    whh = nc.dram_tensor("whh", [128, KT, 4, 128], bf, kind="ExternalInput").ap()
    bhr = nc.dram_tensor("bhr", [128, KT], f32, kind="ExternalInput").ap()
    ident = nc.dram_tensor("ident", [128, 128], bf, kind="ExternalInput").ap()
    bzr = nc.dram_tensor("bzr", [128, 2, KT, B], bf, kind="ExternalInput").ap()
    wo = nc.dram_tensor("wo", [128, KT, O], bf, kind="ExternalInput").ap()
    bo = nc.dram_tensor("bo", [O, 1], f32, kind="ExternalInput").ap()
    yT = nc.dram_tensor("yT", [O, 2, NCH * CH], f32, kind="ExternalOutput").ap()

    with tile.TileContext(nc) as tc, ExitStack() as ctx:
        consts = ctx.enter_context(tc.tile_pool(name="consts", bufs=1))
        big = ctx.enter_context(tc.tile_pool(name="big", bufs=1))
        stg = ctx.enter_context(tc.tile_pool(name="stg", bufs=3))
        gates = ctx.enter_context(tc.tile_pool(name="gates", bufs=2))
        psA = ctx.enter_context(tc.tile_pool(name="psA", bufs=1, space="PSUM"))
        psXP = ctx.enter_context(tc.tile_pool(name="psXP", bufs=2, space="PSUM"))
        psYp = ctx.enter_context(tc.tile_pool(name="psYp", bufs=2, space="PSUM"))

        # ---------- constants ----------
        wcomb_sb = consts.tile([128, MT, 128], bf)
        nc.sync.dma_start(out=wcomb_sb, in_=wcomb)
        bxp_sb = consts.tile([128, MT], f32)
        nc.sync.dma_start(out=bxp_sb, in_=bxp)
        wh8_sb = consts.tile([128, KT, 8, 128], f8)
        nc.sync.dma_start(out=wh8_sb, in_=wh8)
        whh_sb = consts.tile([128, KT, 4, 128], bf)
        nc.sync.dma_start(out=whh_sb, in_=whh)
        bhr_sb = consts.tile([128, KT], f32)
        nc.sync.dma_start(out=bhr_sb, in_=bhr)
        id_sb = consts.tile([128, 128], bf)
        nc.sync.dma_start(out=id_sb, in_=ident)
        bzr_sb = consts.tile([128, 2, KT, B], bf)
        nc.sync.dma_start(out=bzr_sb, in_=bzr)
        wo_sb = consts.tile([128, KT, O], bf)
        nc.sync.dma_start(out=wo_sb, in_=wo)
        bo_sb = consts.tile([O, 1], f32)
        nc.sync.dma_start(out=bo_sb, in_=bo)

        # ---------- per-chain rings ----------
        xps = [[big.tile([128, 4, CH], bf, tag=f"xp{a}{r}", name=f"xp{a}{r}")
                for r in range(3)] for a in range(2)]
        hist = [[big.tile([128, KT, CH], bf, tag=f"hi{a}{r}", name=f"hi{a}{r}")
                 for r in range(3)] for a in range(2)]
        xstg = [[big.tile([128, CH], bf, tag=f"xs{a}{r}", name=f"xs{a}{r}")
                 for r in range(3)] for a in range(2)]
        hz = big.tile([128, KT, B], bf, tag="hz")
        nc.vector.memset(hz, 0.0)

        def xp_mm(a, gemm_slot, m):
            """One xh GEMM matmul + biased copy into chain a's xp ring."""
            ps = psXP.tile([128, CH], f32, tag="xp")
            nc.tensor.matmul(ps, wcomb_sb[:, m, :], xstg[a][gemm_slot],
                             start=True, stop=True)
            if m % 2 == 0:
                nc.scalar.activation(xps[a][gemm_slot][:, m - 8, :], ps,
                                     AF.Identity, bias=bxp_sb[:, m:m + 1],
                                     scale=1.0)
            else:
                nc.vector.tensor_scalar_add(xps[a][gemm_slot][:, m - 8, :], ps,
                                            bxp_sb[:, m:m + 1])

        def step(a, slot, j, h_in):
            """One recurrence step of chain a (full batch, free dim 64)."""
            xp_c = xps[a][slot]
            tk = j * B
            xh = xp_c[:, 0:4, tk:tk + B]
            x_t = xstg[a][slot][:, tk:tk + B]
            h_out = hist[a][slot][:, :, tk:tk + B]

            psRr = psA.tile([128, KT, B], f32, tag="r")
            psZ = psA.tile([128, KT, B], f32, tag="z", bufs=2)
            psH = psA.tile([128, KT, B], f32, tag="h")

            # bias + x-projection into PSUM; no h dependency -- the PE runs
            # these during the other chain's gate chain.
            nc.tensor.matmul(psRr, id_sb, bzr_sb[:, 0], start=True, stop=False)
            nc.tensor.matmul(psZ, id_sb, bzr_sb[:, 1], start=True, stop=False)
            for s in range(KT):
                nc.tensor.matmul(psRr[:, s], wcomb_sb[:, 4 + s, :], x_t,
                                 start=False, stop=False)
                nc.tensor.matmul(psZ[:, s], wcomb_sb[:, s, :], x_t,
                                 start=False, stop=False)
            # r first in its own group (sig_r fires mid-block), then h, z last.
            for s in range(KT):
                for k in range(KT):
                    nc.tensor.matmul(psRr[:, s], wh8_sb[:, k, 4 + s, :],
                                     h_in[:, k], start=False,
                                     stop=(s == KT - 1 and k == KT - 1))
            for s in range(KT):
                for k in range(KT):
                    nc.tensor.matmul(psH[:, s], whh_sb[:, k, s, :],
                                     h_in[:, k], start=(k == 0),
                                     stop=(k == KT - 1))
            for s in range(KT):
                for k in range(KT):
                    nc.tensor.matmul(psZ[:, s], wh8_sb[:, k, s, :],
                                     h_in[:, k], start=False,
                                     stop=(s == KT - 1 and k == KT - 1))

            r_sb = gates.tile([128, KT, B], bf, tag="r")
            nc.scalar.activation(r_sb, psRr, AF.Sigmoid)
            z_sb = gates.tile([128, KT, B], bf, tag="z")
            nc.scalar.activation(z_sb, psZ, AF.Sigmoid)
            t1 = gates.tile([128, KT, B], bf, tag="t1")
            if has_bh:
                for s in range(KT):
                    nc.vector.scalar_tensor_tensor(
                        t1[:, s], psH[:, s], bhr_sb[:, s:s + 1], r_sb[:, s],
                        OP.add, OP.mult)
            else:
                nc.vector.tensor_mul(t1, psH, r_sb)
            t2 = gates.tile([128, KT, B], bf, tag="t2")
            nc.vector.tensor_add(t2, t1, xh)
            hh = gates.tile([128, KT, B], bf, tag="hh")
            nc.scalar.activation(hh, t2, AF.Tanh)
            dd = gates.tile([128, KT, B], bf, tag="dd")
            nc.vector.tensor_sub(dd, h_in, hh)
            ee = gates.tile([128, KT, B], bf, tag="ee")
            nc.vector.tensor_mul(ee, z_sb, dd)
            nc.vector.tensor_add(h_out, ee, hh)
            return h_out

        def dual_chunk(c_slot, prev_slot, gemm_slot, hps, nsteps=SPC,
                       first_chunk=False):
            """nsteps x 2 chains, interleaved; xp GEMM as PE gap filler."""
            for a in range(2):
                if first_chunk:
                    hps[a] = hz
                elif hps[a] is None:
                    hps[a] = hist[a][prev_slot][:, :, (SPC - 1) * B:SPC * B]
            for j in range(nsteps):
                hps[0] = step(0, c_slot, j, hps[0])
                hps[1] = step(1, c_slot, j, hps[1])
                if gemm_slot is not None:
                    for m, a in M_SCHED[j]:
                        xp_mm(a, gemm_slot, m)
            return hps

        def y_gemm(a, c_slot, c_expr):
            psY = psYp.tile([O, CH], f32, tag="y")
            for k in range(KT):
                nc.tensor.matmul(psY, wo_sb[:, k, :], hist[a][c_slot][:, k, :],
                                 start=(k == 0), stop=(k == KT - 1))
            yst = stg.tile([O, CH], f32, tag="yst")
            nc.scalar.activation(yst, psY, AF.Identity, bias=bo_sb, scale=1.0)
            nc.sync.dma_start(out=yT[:, a, ds(c_expr * CH, CH)], in_=yst)

        # ---------- prologue: xp chunks 0/1 both chains, chunk 0 steps ----
        for a in range(2):
            nc.sync.dma_start(out=xstg[a][0], in_=xT[:, a, 0:CH])
            nc.sync.dma_start(out=xstg[a][1], in_=xT[:, a, CH:2 * CH])
        for a in range(2):
            for m in range(8, MT):
                xp_mm(a, 0, m)
            for m in range(8, MT):
                xp_mm(a, 1, m)
        for a in range(2):
            nc.sync.dma_start(out=xstg[a][2], in_=xT[:, a, 2 * CH:3 * CH])
        hps = [None, None]
        hps = dual_chunk(0, None, 2, hps, first_chunk=True)
        y_gemm(0, 0, 0)
        y_gemm(1, 0, 0)

        # ---------- main chunks 1..6, fully unrolled ----------
        for i in (1, 4):
            # chunk i -> slot 1, i+1 -> slot 2, i+2 -> slot 0
            for a in range(2):
                nc.sync.dma_start(out=xstg[a][0],
                                  in_=xT[:, a, ds((i + 2) * CH, CH)])
            hps = dual_chunk(1, 0, 0, [None, None])
            y_gemm(0, 1, i)
            y_gemm(1, 1, i)
            for a in range(2):
                nc.sync.dma_start(out=xstg[a][1],
                                  in_=xT[:, a, ds((i + 3) * CH, CH)])
            hps = dual_chunk(2, 1, 1, [None, None])
            y_gemm(0, 2, i + 1)
            y_gemm(1, 2, i + 1)
            for a in range(2):
                nc.sync.dma_start(out=xstg[a][2],
                                  in_=xT[:, a, ds((i + 4) * CH, CH)])
            hps = dual_chunk(0, 2, 2, [None, None])
            y_gemm(0, 0, i + 2)
            y_gemm(1, 0, i + 2)

        # ---------- epilogue: chunk 7 (full), 8 (partial) ----------
        for a in range(2):
            nc.sync.dma_start(out=xstg[a][0], in_=xT[:, a, 9 * CH:10 * CH])
        hps = dual_chunk(1, 0, 0, [None, None])       # chunk 7, gemm c9 pad
        y_gemm(0, 1, 7)
        y_gemm(1, 1, 7)
        hps = dual_chunk(2, 1, None, [None, None], nsteps=SPC_LAST)  # chunk 8
        y_gemm(0, 2, 8)
        y_gemm(1, 2, 8)

    nc.compile()
    return nc


def _get_program(has_bh: bool):
    key = ("prog", has_bh)
    if key not in _cache:
        _cache[key] = _build(has_bh)
    return _cache[key]


def _chain_tokens(x, dirn, seg):
    """[128, NCHP*CH] bf16 token stream for one sub-segment chain."""
    t0 = seg * L8
    tsel = np.arange(t0, t0 + P)
    tglob = tsel if dirn == 0 else (T - 1 - tsel)
    xs = x[:, tglob, :]                                   # [B,P,F]
    out = np.zeros((128, NCHP * CH), np.float32)
    out[:, :CTOK] = xs.transpose(2, 1, 0).reshape(F, CTOK)
    return out


def _prep_core(x, dirn, i, wcomb_bf, bxp_f, wh, bb, wo_half, bias_out):
    """Per-core input map. x is the full [B,T,F] fp32 array."""
    xTc = np.stack([_chain_tokens(x, dirn, 2 * i),
                    _chain_tokens(x, dirn, 2 * i + 1)], axis=1)
    whr = wh.reshape(KT, 128, MT, 128).transpose(1, 0, 2, 3)  # [kp,ks,m,p]
    return {
        "xT": xTc.astype(BF16),
        "wcomb": wcomb_bf,
        "bxp": bxp_f,
        "wh8": np.ascontiguousarray(whr[:, :, 0:8]).astype(FP8),
        "whh": np.ascontiguousarray(whr[:, :, 8:12]).astype(BF16),
        "bhr": np.ascontiguousarray(
            bb[1, 2 * H:].reshape(KT, 128).T.astype(np.float32)),
        "ident": np.eye(128).astype(BF16),
        "bzr": np.ascontiguousarray(np.broadcast_to(
            np.stack([bxp_f[:, 4:8], bxp_f[:, 0:4]], axis=1)[:, :, :, None],
            (128, 2, KT, B))).astype(BF16),
        "wo": np.ascontiguousarray(
            wo_half.reshape(KT, 128, O).transpose(1, 0, 2)).astype(BF16),
        "bo": bias_out.reshape(O, 1).astype(np.float32),
    }


def _prepare(np_in):
    """Build (nc, in_maps) for the 8 cores."""
    s1 = np_in["g1"] / np.sqrt(np_in["v1"] + EPS)
    b1 = (np_in["b_in"] - np_in["m1"]) * s1 + np_in["be1"]
    s2 = np_in["g2"] / np.sqrt(np_in["v2"] + EPS)
    b2 = (np_in["b_out"] - np_in["m2"]) * s2 + np_in["be2"]
    Ws = np_in["w_out"] * s2[None, :]

    has_bh = bool(np.any(np_in["bf"][1, 2 * H:]) or np.any(np_in["bb"][1, 2 * H:]))
    nc = _get_program(has_bh)

    in_maps = []
    for c in range(NCORES):
        dirn, i = c // 4, c % 4
        wx = np_in["wxf"] if dirn == 0 else np_in["wxb"]
        wh = np_in["whf"] if dirn == 0 else np_in["whb"]
        bb = np_in["bf"] if dirn == 0 else np_in["bb"]
        wcomb = ((np_in["w_in"] * s1[None, :]) @ wx).astype(np.float32)
        wcomb_bf = np.ascontiguousarray(
            wcomb.reshape(128, MT, 128)).astype(BF16)
        bxp_full = (b1 @ wx + bb[0]
                    + np.concatenate([bb[1, :2 * H], np.zeros(H, np.float32)]))
        bxp_f = np.ascontiguousarray(
            bxp_full.reshape(MT, 128).T.astype(np.float32))
        wo_half = Ws[:H] if dirn == 0 else Ws[H:]
        bias_o = b2 if dirn == 0 else np.zeros(O, np.float32)
        in_maps.append(_prep_core(np_in["x"], dirn, i, wcomb_bf, bxp_f,
                                  wh, bb, wo_half, bias_o))
    return nc, in_maps


def _assemble(outs):
    """Sum per-core yT partials into the full [B,T,O] output."""
    y = np.zeros((B, T, O), np.float32)
    for c in range(NCORES):
        dirn, i = c // 4, c % 4
        yc = outs[c]["yT"].reshape(O, 2, NCH * CH)
        for a in range(2):
            seg = 2 * i + a
            t0 = seg * L8
            tsel = np.arange(t0, t0 + P)
            tglob = tsel if dirn == 0 else (T - 1 - tsel)
            k0 = 0 if seg == 0 else W
            ya = yc[:, a, :CTOK].reshape(O, P, B)
            y[:, tglob[k0:], :] += ya[:, k0:, :].transpose(2, 1, 0)
    return y


def kernel(x, w_in, b_in, g1, be1, m1, v1, wxf, whf, bf, wxb, whb, bb,
           w_out, b_out, g2, be2, m2, v2):
    from concourse.bass_utils import run_bass_kernel_spmd

    args = locals()
    np_in = {k: np.asarray(args[k], np.float32) for k in (
        "x", "w_in", "b_in", "g1", "be1", "m1", "v1", "wxf", "whf", "bf",
        "wxb", "whb", "bb", "w_out", "b_out", "g2", "be2", "m2", "v2")}
    nc, in_maps = _prepare(np_in)
    res = run_bass_kernel_spmd(nc, in_maps, core_ids=list(range(NCORES)))
    return _assemble(res.results)
